# revision 1
# baseline (speedup 1.0000x reference)
import sys

for p in ("/opt/trn_rl_repo",):
    if p not in sys.path:
        sys.path.insert(0, p)

import numpy as np
import ml_dtypes

import concourse.bass as bass
import concourse.mybir as mybir
import concourse.tile as tile
from concourse import bacc, bass_utils
from concourse.kernels.tile_matmul import matmul_tile_kernel

# Problem dims (hardcoded per contract)
B, S, DM, H, Dh = 2, 4096, 2048, 16, 128
NCORES = 8
SL = (B * S) // NCORES      # 1024 positions per core
P = 128
KT = DM // P                # 16 contraction tiles
MT = SL // P                # 8 m-tiles

_BF16 = ml_dtypes.bfloat16


def _build_nc():
    """Per-core kernel: Q/K/V = x_shard @ W.T via production matmul.

    kxm = x^T  [P, KT, SL]  (contraction e on partitions)
    kxn = W^T  [P, KT, DM]
    mxn = out  [P, MT, DM]  fp32
    """
    nc = bacc.Bacc(None, target_bir_lowering=False)
    xkm = nc.dram_tensor("xkm", [P, KT, SL], mybir.dt.bfloat16, kind="ExternalInput")
    wts = [
        nc.dram_tensor(f"w{n}", [P, KT, DM], mybir.dt.bfloat16, kind="ExternalInput")
        for n in ("q", "k", "v")
    ]
    outs = [
        nc.dram_tensor(f"{n}o", [P, MT, DM], mybir.dt.float32, kind="ExternalOutput")
        for n in ("q", "k", "v")
    ]
    with tile.TileContext(nc) as tc:
        for w, o in zip(wts, outs):
            matmul_tile_kernel(tc, xkm[:], w[:], o[:])
    nc.finalize()
    return nc


_NC_CACHE = None


def _get_nc():
    global _NC_CACHE
    if _NC_CACHE is None:
        _NC_CACHE = _build_nc()
    return _NC_CACHE


def _to_kpm(a2d):
    """[K, M] row-major -> [P, K//P, M] (p k m) with p innermost of K."""
    K, M = a2d.shape
    return np.ascontiguousarray(
        a2d.reshape(K // P, P, M).transpose(1, 0, 2)
    )


def kernel(x, Wq, bq, Wk, bk, Wv, bv):
    x = np.asarray(x, dtype=np.float32)
    xf = np.ascontiguousarray(x.reshape(B * S, DM))

    ws = []
    for W in (Wq, Wk, Wv):
        wT = np.asarray(W, np.float32).T.astype(_BF16)   # [e, f]
        ws.append(_to_kpm(wT))

    in_maps = []
    for c in range(NCORES):
        shard = xf[c * SL:(c + 1) * SL, :]               # [SL, DM]
        xT = shard.T.astype(_BF16)                       # [e, s]
        in_maps.append({
            "xkm": _to_kpm(xT), "wq": ws[0], "wk": ws[1], "wv": ws[2],
        })

    nc = _get_nc()
    res = bass_utils.run_bass_kernel_spmd(nc, in_maps, core_ids=list(range(NCORES)))
    results = res.results

    def gather(name):
        # [P, MT, DM] per core -> [SL, DM] -> concat cores -> [B*S, DM]
        return np.concatenate(
            [r[name].transpose(1, 0, 2).reshape(SL, DM) for r in results], axis=0
        )

    Q = gather("qo") + np.asarray(bq, np.float32)
    K = gather("ko") + np.asarray(bk, np.float32)
    V = gather("vo") + np.asarray(bv, np.float32)

    Q = Q.reshape(B * S, H, Dh)
    K = K.reshape(B * S, H, Dh)
    V = V.reshape(B * S, H, Dh)

    # Per-position attention over the HEADS axis (faithful to reference)
    scores = np.matmul(Q, K.transpose(0, 2, 1)) / np.sqrt(Dh)  # [BS, H, H]
    scores -= scores.max(axis=-1, keepdims=True)
    np.exp(scores, out=scores)
    scores /= scores.sum(axis=-1, keepdims=True)
    out = np.matmul(scores, V)                                  # [BS, H, Dh]

    # reference: [B,S,H,D] -> transpose(0,2,1,3) -> reshape(B,S,H*D)
    out = out.reshape(B, S, H, Dh).transpose(0, 2, 1, 3).reshape(B, S, H * Dh)
    return np.ascontiguousarray(out.astype(np.float32))



# revision 10
# speedup vs baseline: 4.2429x; 4.2429x over previous
import sys

for p in ("/opt/trn_rl_repo",):
    if p not in sys.path:
        sys.path.insert(0, p)

import numpy as np
import ml_dtypes

import concourse.bass as bass
import concourse.mybir as mybir
import concourse.tile as tile
from concourse import bacc, bass_utils

# Problem dims (hardcoded per contract)
B, S, DM, H, D = 2, 4096, 2048, 16, 128
NCORES = 8
SL = (B * S) // NCORES          # 1024 positions per core
P = 128
KT = DM // P                    # 16 contraction k-tiles
NG = SL // 8                    # 128 groups of 8 positions per core
SCALE = 1.0 / float(np.sqrt(D))

_BF16 = ml_dtypes.bfloat16
BF = mybir.dt.bfloat16
F32 = mybir.dt.float32


def _build_nc(n_cores):
    """Fused per-core kernel.

    Inputs (per core):
      xkm  [128, 16, 1024] bf16  -- x^T shard (e on partitions, e = k*128+p)
      bias [3, 2048]       f32   -- bq, bk, bv
      wsh  [3, 128, 2, 2048] bf16 (8-core) -- this core's 2 k-tiles of
           Wq^T/Wk^T/Wv^T in kpm layout; AllGathered on device.
      (1-core sim build takes full weights wgq/wgk/wgv [8,128,2,2048].)

    Output (per core):
      out [16, 64, 16, 128] bf16 = attn_out[h, g, j, d] for local position
          s = 16*g + j -- matches the reference's scrambled
          transpose(0,2,1,3).reshape layout after host concat.

    Compute: Q/K/V projections (+bias via an extra augmented k-tile), then
    per-position attention over the HEADS axis using block-diagonal 8-position
    PE matmuls, softmax over heads, weighted V sum, normalized at the end.
    """
    nc = bacc.Bacc(None, target_bir_lowering=False, num_devices=n_cores)

    xkm = nc.dram_tensor("xkm", [P, KT, SL], BF, kind="ExternalInput")
    bias_d = nc.dram_tensor("bias", [3, DM], BF, kind="ExternalInput")
    out_d = nc.dram_tensor("out", [H, SL // 16, 16, D], BF, kind="ExternalOutput")

    if n_cores == 1:
        wg = [
            nc.dram_tensor(f"wg{n}", [NCORES, P, 2, DM], BF, kind="ExternalInput")
            for n in "qkv"
        ]
        wsh = None
    else:
        wsh = nc.dram_tensor("wsh", [3, P, 2, DM], BF, kind="ExternalInput")
        wg = [nc.dram_tensor(f"wg{n}", [NCORES, P, 2, DM], BF) for n in "qkv"]
        wb = [nc.dram_tensor(f"wb{n}", [P, 2, DM], BF) for n in "qkv"]

    with tile.TileContext(nc) as tc:
        with (
            tc.tile_pool(name="const", bufs=1) as const,
            tc.tile_pool(name="xpool", bufs=1) as xpool,
            tc.tile_pool(name="mega", bufs=1) as mega,
            tc.tile_pool(name="vpool", bufs=1) as vpool,
            tc.tile_pool(name="vstkp", bufs=1) as vstkp,
            tc.tile_pool(name="wstream", bufs=2) as wstream,
            tc.tile_pool(name="pspool", bufs=4, space="PSUM") as pspool,
            tc.tile_pool(name="psA", bufs=2, space="PSUM") as psA_pool,
            tc.tile_pool(name="psO", bufs=2, space="PSUM") as psO_pool,
            tc.tile_pool(name="attn", bufs=3) as attn,
        ):
            # ---- weight all-gather (v first: the V phase needs it first) ----
            if n_cores > 1:
                for i, n in ((2, "v"), (0, "q"), (1, "k")):
                    nc.sync.dma_start(out=wb[i][:], in_=wsh[i])
                    nc.gpsimd.collective_compute(
                        "AllGather",
                        mybir.AluOpType.bypass,
                        replica_groups=[list(range(n_cores))],
                        ins=[wb[i][:].opt()],
                        outs=[wg[i][:].opt()],
                    )

            # ---- x^T load + augmented ones k-tile (for bias matmuls) ----
            xa = xpool.tile([P, (KT + 1) * SL], BF)
            nc.sync.dma_start(
                out=xa[:, 0 : KT * SL], in_=xkm[:].rearrange("p a b -> p (a b)")
            )
            nc.vector.memset(xa[:, KT * SL :], 0.0)
            nc.vector.memset(xa[0:1, KT * SL :], 1.0)

            # ---- bias row tiles: [128, 2048] bf16, row 0 = bias, rest 0 ----
            bsb = []
            for i in range(3):
                bt = const.tile([P, DM], BF, tag=f"bias{i}")
                nc.vector.memset(bt[:], 0.0)
                nc.sync.dma_start(out=bt[0:1, :], in_=bias_d[i : i + 1, :])
                bsb.append(bt)

            # ---- constants for attention ----
            mask_np = np.full((P, P), -30000.0, np.float32)
            for b8 in range(8):
                mask_np[16 * b8 : 16 * b8 + 16, 16 * b8 : 16 * b8 + 16] = 0.0
            mask_dram = nc.inline_tensor(mask_np, name="maskneg_c")
            maskneg = const.tile([P, P], F32)
            nc.sync.dma_start(out=maskneg[:], in_=mask_dram[:])
            ones_col = const.tile([P, 1], BF)
            nc.vector.memset(ones_col[:], 1.0)

            # =================== Phase B: V projection ===================
            # V in natural layout [s-part, f], Wv^T resident in SBUF.
            wv_sb = mega.tile([P, KT * DM], BF, tag="mega")
            nc.sync.dma_start(
                out=wv_sb[:].rearrange("p (a b f) -> p a b f", a=NCORES, b=2),
                in_=wg[2][:].rearrange("a p b f -> p a b f"),
            )
            v_sb = vpool.tile([P, (SL // P) * DM], BF)
            vstack = vstkp.tile([P, NG * P], BF)
            for m in range(SL // P):
                for ns in range(DM // 512):
                    ps = pspool.tile([P, 512], F32)
                    for ke in range(KT):
                        nc.tensor.matmul(
                            ps[:],
                            xa[:, ke * SL + m * P : ke * SL + (m + 1) * P],
                            wv_sb[:, ke * DM + ns * 512 : ke * DM + (ns + 1) * 512],
                            start=(ke == 0),
                            stop=False,
                        )
                    nc.tensor.matmul(
                        ps[:],
                        xa[:, KT * SL + m * P : KT * SL + (m + 1) * P],
                        bsb[2][:, ns * 512 : (ns + 1) * 512],
                        start=False,
                        stop=True,
                    )
                    nc.vector.tensor_scalar_add(
                        v_sb[:, m * DM + ns * 512 : m * DM + (ns + 1) * 512],
                        ps[:],
                        0.0,
                    )
                # interleave this m-tile into vstack group tiles:
                # vstack[:, g*128+16*sl+t? ...] -- per group g: partition
                # p = 16*s_l + t holds V[8g+s_l, t*128:(t+1)*128]
                for gl in range(16):
                    g = m * 16 + gl
                    nc.sync.dma_start(
                        out=vstack[:, g * P : (g + 1) * P],
                        in_=v_sb[
                            8 * gl : 8 * gl + 8, m * DM : (m + 1) * DM
                        ].rearrange("p (t d) -> p t d", t=16),
                    )

            # =================== Phase C: Q^T / K^T projections ===================
            # Transposed layout: qt/kt [d-part, s, h] (s-major, h-minor free).
            qtkt = mega.tile([P, 2 * SL * H], BF, tag="mega")
            for w_i, base in ((0, 0), (1, SL * H)):
                for f in range(H):  # f-tile == head
                    wt = wstream.tile([P, KT * P], BF, tag="wt")
                    for kp in range(NCORES):
                        nc.sync.dma_start(
                            out=wt[:, kp * 2 * P : (kp + 1) * 2 * P].rearrange(
                                "p (b f) -> p b f", b=2
                            ),
                            in_=wg[w_i][kp, :, :, f * P : (f + 1) * P],
                        )
                    for ns in range(SL // 512):
                        ps = pspool.tile([P, 512], F32)
                        for ke in range(KT):
                            nc.tensor.matmul(
                                ps[:],
                                wt[:, ke * P : (ke + 1) * P],
                                xa[:, ke * SL + ns * 512 : ke * SL + (ns + 1) * 512],
                                start=(ke == 0),
                                stop=False,
                            )
                        nc.tensor.matmul(
                            ps[:],
                            bsb[w_i][:, f * P : (f + 1) * P],
                            xa[:, KT * SL + ns * 512 : KT * SL + (ns + 1) * 512],
                            start=False,
                            stop=True,
                        )
                        # strided evict: qt[d, s, h]: col = s*H + h
                        nc.scalar.copy(
                            qtkt[:, base : base + SL * H].rearrange(
                                "p (s h) -> p s h", h=H
                            )[:, ns * 512 : (ns + 1) * 512, f],
                            ps[:],
                        )

            # =================== Phase D: attention ===================
            qt = qtkt[:, 0 : SL * H]
            kt = qtkt[:, SL * H : 2 * SL * H]
            out4 = out_d[:]
            for g in range(NG):
                psa = psA_pool.tile([P, P], F32)
                nc.tensor.matmul(
                    psa[:],
                    kt[:, g * P : (g + 1) * P],
                    qt[:, g * P : (g + 1) * P],
                    start=True,
                    stop=True,
                )
                sc = attn.tile([P, P], F32, tag="sc")
                # sc = psa * (1/sqrt(D)) + maskneg
                nc.vector.scalar_tensor_tensor(
                    sc[:],
                    psa[:],
                    SCALE,
                    maskneg[:],
                    mybir.AluOpType.mult,
                    mybir.AluOpType.add,
                )
                ex = attn.tile([P, P], BF, tag="ex")
                nc.scalar.activation(
                    ex[:], sc[:], mybir.ActivationFunctionType.Exp
                )
                pso = psO_pool.tile([P, P + 4], F32)
                nc.tensor.matmul(
                    pso[:, 0:P], ex[:], vstack[:, g * P : (g + 1) * P],
                    start=True, stop=True,
                )
                nc.tensor.matmul(
                    pso[:, P : P + 1], ex[:], ones_col[:], start=True, stop=True
                )
                zrec = attn.tile([P, 1], F32, tag="zrec")
                nc.vector.reciprocal(zrec[:], pso[:, P : P + 1])
                ost = attn.tile([P, P], BF, tag="ost")
                nc.vector.tensor_scalar_mul(ost[:], pso[:, 0:P], zrec[:])
                nc.sync.dma_start(
                    out=out4[:, g // 2, 8 * (g % 2) : 8 * (g % 2) + 8, :].rearrange(
                        "h j d -> j h d"
                    ),
                    in_=ost[:],
                )

    nc.finalize()
    return nc


_NC_CACHE = {}


def _get_nc(n_cores=NCORES):
    if n_cores not in _NC_CACHE:
        _NC_CACHE[n_cores] = _build_nc(n_cores)
    return _NC_CACHE[n_cores]


def _prep_x(x):
    """x [2,4096,2048] f32 -> [8, 128, 16, 1024] bf16 (per-core x^T kpm)."""
    x = np.asarray(x, np.float32)
    return x.reshape(NCORES, SL, KT, P).transpose(0, 3, 2, 1).astype(_BF16)


def _prep_w(W):
    """W [2048,2048] f32 -> W^T kpm [128, 16, 2048] bf16."""
    return np.asarray(W, np.float32).reshape(DM, KT, P).transpose(2, 1, 0).astype(_BF16)


def _make_inmaps(inputs):
    xall = _prep_x(inputs["x"])
    ws = [_prep_w(inputs[n]) for n in ("Wq", "Wk", "Wv")]
    bias = np.ascontiguousarray(
        np.stack(
            [np.asarray(inputs[n], np.float32) for n in ("bq", "bk", "bv")]
        ).astype(_BF16)
    )
    in_maps = []
    for c in range(NCORES):
        wsh = np.ascontiguousarray(
            np.stack([w[:, 2 * c : 2 * c + 2, :] for w in ws])
        )
        in_maps.append({"xkm": np.ascontiguousarray(xall[c]), "wsh": wsh, "bias": bias})
    return in_maps


def _assemble(results):
    full = np.empty((B, H, S // 16, DM), np.float32)
    for c, r in enumerate(results):
        full[c // 4, :, (c % 4) * 64 : (c % 4 + 1) * 64, :] = r["out"].reshape(
            H, SL // 16, DM
        )
    return np.ascontiguousarray(full.reshape(B, S, DM))


def kernel(x, Wq, bq, Wk, bk, Wv, bv):
    in_maps = _make_inmaps(
        {"x": x, "Wq": Wq, "bq": bq, "Wk": Wk, "bk": bk, "Wv": Wv, "bv": bv}
    )
    nc = _get_nc(NCORES)
    res = bass_utils.run_bass_kernel_spmd(nc, in_maps, core_ids=list(range(NCORES)))
    return _assemble(res.results)
